# revision 32
# baseline (speedup 1.0000x reference)
"""Cross-attention kernel for TRN2 (8 NeuronCores, data-parallel over batch).

Problem (per batch element b):
    s[e,t] = sum_d enc[b,e,d] * dec[b,t,d]
    a      = softmax(s, axis=e)
    out[b,t,d] = sum_e a[e,t] * enc[b,e,d]

Per-core layout (B=8 -> one batch element per core):
  - mm1 computes s in [t_block=128, e] layout: lhsT = decT (d-major), rhs =
    encT (d-major); contraction over d on the PE partition axis.
  - softmax along the free axis: DVE reduce_max(negate) -> ACT exp with
    per-partition bias and accumulated row sum Z.
  - p is transposed 128x128-wise on the PE (identity matmul) so the second
    matmul can contract over e; mm2: lhsT = pT, rhs = enc (natural layout).
  - 1/Z is applied during PSUM evacuation of mm2 (per-partition scalar mul).

Host side transposes enc/dec once (numpy) so the device never transposes
inputs.
"""

import numpy as np

import concourse.bass as bass
import concourse.tile as tile
from concourse import masks, mybir
from concourse.bass_utils import run_bass_kernel_spmd

F32 = mybir.dt.float32
F32R = mybir.dt.float32r

B, S_ENC, S_DEC, D = 8, 2048, 2048, 512
N_CORES = 8

# Matmul input precision knobs (F32 = exact, F32R = ~1e-4, 4x faster rows)
MM1_DT = F32R
MM2_DT = F32R


def _split_multi_waits(nc):
    """This walrus build rejects any instruction with >1 sync wait. Hoist
    surplus waits onto single-wait same-engine NOPs placed just before."""
    for f in nc.m.functions:
        for bb in f.blocks:
            new_list = []
            changed = False
            for inst in bb.instructions:
                si = inst.sync_info
                waits = list(si.on_wait) if si and si.on_wait else []
                if len(waits) > 1:
                    changed = True
                    for w in waits[:-1]:
                        nop = mybir.InstNoOp(
                            name=nc.get_next_instruction_name(),
                            engine=inst.engine,
                            sync_info=mybir.SyncInfo(on_wait=[w], on_update=[]),
                            bass_nofuse=True,
                        )
                        nc.register_instruction(nop, overwrite=True)
                        new_list.append(nop)
                    si.on_wait = waits[-1:]
                new_list.append(inst)
            if changed:
                bb.instructions = new_list


def attention_body(tc, out, encT, decT, enc, E, T, Dd, mm1_dt, mm2_dt, dbg=None):
    nc = tc.nc
    KD = Dd // 128   # d-tiles (contraction of mm1)
    NE = E // 512    # e-chunks of mm1 output (psum bank-sized)
    JT = E // 128    # e-tiles (contraction of mm2 / transposes)
    TB = T // 128    # t row-blocks
    Exp = mybir.ActivationFunctionType.Exp
    X = mybir.AxisListType.X

    with (
        tc.tile_pool(name="resident", bufs=1) as res_pool,
        tc.tile_pool(name="work", bufs=2) as work,
        tc.tile_pool(name="ps_s", bufs=1, space="PSUM") as ps_s,
        tc.tile_pool(name="ps_t", bufs=4, space="PSUM") as ps_t,
        tc.tile_pool(name="ps_c", bufs=2, space="PSUM") as ps_c,
    ):
        encTt = res_pool.tile([128, KD, E], mm1_dt)
        decTt = res_pool.tile([128, KD, T], mm1_dt)
        encS = res_pool.tile([128, JT, Dd], mm2_dt)
        ident = res_pool.tile([128, 128], mm2_dt)

        # Inputs are declared in the matmul dtype directly (f32r binds fp32
        # bits; the PE rounds internally) -> plain DMAs, no rounding casts.
        # Load order matters for the startup ramp: mm1 block 0 needs ALL of
        # encT but only the first t-columns of decT; encS is needed only by
        # mm2 (one softmax later). So: encT, then decT in t-chunks
        # (earliest first), then encS.
        for k in range(KD):
            nc.gpsimd.dma_start(encTt[:, k, :], encT[k * 128:(k + 1) * 128, :])
        TC = T // 4
        for c_ in range(4):
            for k in range(KD):
                nc.gpsimd.dma_start(decTt[:, k, c_ * TC:(c_ + 1) * TC],
                                    decT[k * 128:(k + 1) * 128, c_ * TC:(c_ + 1) * TC])
        enc_r = enc.rearrange("(g p) d -> p g d", p=128)
        GJ = JT // 4
        for g in range(4):
            nc.gpsimd.dma_start(encS[:, g * GJ:(g + 1) * GJ, :],
                                enc_r[:, g * GJ:(g + 1) * GJ, :])
        if mm2_dt == F32:
            masks.make_identity(nc, ident[:])
        else:
            identf = res_pool.tile([128, 128], F32)
            masks.make_identity(nc, identf[:])
            nc.vector.tensor_copy(ident[:], identf[:])

        state = None
        for tb in range(TB + 1):
            cur = None
            if tb < TB:
                # mm1 in two e-halves with online softmax: the half-size
                # score psum (2 banks) frees 2 PSUM banks for transpose
                # slots (ps_t bufs=4), which removes the per-block PE stall
                # on pT-copy slot recycling.
                EH = E // 2
                NH = NE // 2
                p = work.tile([128, E], mm2_dt, tag="p")
                pmax = work.tile([128, NE], F32, tag="pmax")
                zp = work.tile([128, NE], F32, tag="zp")
                negmA = work.tile([128, 1], F32, tag="negmA")
                negM = work.tile([128, 1], F32, tag="negM")
                for h in range(2):
                    psum_s = ps_s.tile([128, EH], F32, tag="s")
                    for k in range(KD):
                        for n2 in range(NH):
                            n = h * NH + n2
                            nc.tensor.matmul(
                                psum_s[:, n2 * 512:(n2 + 1) * 512],
                                decTt[:, k, tb * 128:(tb + 1) * 128],
                                encTt[:, k, n * 512:(n + 1) * 512],
                                start=(k == 0),
                                stop=(k == KD - 1),
                            )
                    for n2 in range(NH):
                        n = h * NH + n2
                        nc.vector.reduce_max(out=pmax[:, n:n + 1],
                                             in_=psum_s[:, n2 * 512:(n2 + 1) * 512],
                                             axis=X)
                    nm = negmA if h == 0 else negM
                    nc.vector.reduce_max(out=nm[:], in_=pmax[:, h * NH:(h + 1) * NH],
                                         axis=X, negate=True)
                    if h == 1:
                        # negM = -max(mA, mB) = min(negmA, negmB)
                        nc.vector.tensor_tensor(negM[:], negM[:], negmA[:],
                                                op=mybir.AluOpType.min)
                    bias = negmA if h == 0 else negM
                    for n2 in range(NH):
                        n = h * NH + n2
                        nc.scalar.activation(out=p[:, n * 512:(n + 1) * 512],
                                             in_=psum_s[:, n2 * 512:(n2 + 1) * 512],
                                             func=Exp, bias=bias[:], scale=1.0,
                                             accum_out=zp[:, n:n + 1])
                # rescale half A by f = exp(mA - M) = exp(negM - negmA)
                fsc = work.tile([128, 1], F32, tag="fsc")
                nc.vector.tensor_sub(fsc[:], negM[:], negmA[:])
                nc.scalar.activation(out=fsc[:], in_=fsc[:], func=Exp,
                                     bias=0.0, scale=1.0)
                nc.vector.tensor_scalar_mul(p[:, 0:EH], p[:, 0:EH], fsc[:])
                # Z = (zA0+zA1)*f + (zB0+zB1)
                zA = work.tile([128, 1], F32, tag="zA")
                nc.vector.reduce_sum(out=zA[:], in_=zp[:, 0:NH], axis=X)
                zB = work.tile([128, 1], F32, tag="zB")
                nc.vector.reduce_sum(out=zB[:], in_=zp[:, NH:NE], axis=X)
                z = work.tile([128, 1], F32, tag="z")
                nc.vector.tensor_scalar(out=z[:], in0=zA[:], scalar1=fsc[:],
                                        scalar2=zB[:], op0=mybir.AluOpType.mult,
                                        op1=mybir.AluOpType.add)
                rz = work.tile([128, 1], F32, tag="rz")
                nc.vector.reciprocal(rz[:], z[:])
                cur = (p, rz, tb)

            if state is not None:
                pp, rz, tbp = state
                pT = work.tile([128, JT, 128], mm2_dt, tag="pT")
                for j in range(JT):
                    pst = ps_t.tile([128, 128], mm2_dt, tag="pt")
                    nc.tensor.transpose(pst[:], pp[:, j * 128:(j + 1) * 128], ident[:])
                    if j % 2 == 0:
                        nc.scalar.copy(pT[:, j, :], pst[:])
                    else:
                        nc.vector.tensor_copy(pT[:, j, :], pst[:])
                psum_c = ps_c.tile([128, Dd], F32, tag="c")
                for j in range(JT):
                    nc.tensor.matmul(psum_c[:], pT[:, j, :], encS[:, j, :],
                                     start=(j == 0), stop=(j == JT - 1))
                c = work.tile([128, Dd], F32, tag="c_sb")
                nc.vector.tensor_scalar_mul(c[:], psum_c[:], rz[:])
                nc.gpsimd.dma_start(out[tbp * 128:(tbp + 1) * 128, :], c[:])

            state = cur


def attention_body_b(tc, out, encT, decT, enc, E, T, Dd, mm1_dt, mm2_dt, dbg=None):
    """Plan B (unused alternative): scores in [e,t] layout, no probability
    transposes. True per-row max via PE transposes of the folded max + a
    rank-1 ones matmul broadcast; Z via fold-add + PE transpose + reduce.
    Measured slower than Plan A on this HW (227.5us vs 224.8us): every
    matmul changes weights and the folds load the DVE heavily.
    """
    nc = tc.nc
    KD = Dd // 128   # d-tiles (contraction of mm1)
    JT = E // 128    # e-tiles
    WB = T // 512    # t column-blocks
    Exp = mybir.ActivationFunctionType.Exp
    X = mybir.AxisListType.X

    with (
        tc.tile_pool(name="resident", bufs=1) as res_pool,
        tc.tile_pool(name="work", bufs=2) as work,
        tc.tile_pool(name="small", bufs=2) as small,
        tc.tile_pool(name="ps_s", bufs=4, space="PSUM") as ps_s,
        tc.tile_pool(name="ps_m", bufs=1, space="PSUM") as ps_m,
        tc.tile_pool(name="ps_c", bufs=2, space="PSUM") as ps_c,
    ):
        encTt = res_pool.tile([128, KD, E], mm1_dt)
        decTt = res_pool.tile([128, KD, T], mm1_dt)
        encS = res_pool.tile([128, JT, Dd], mm2_dt)
        identf = res_pool.tile([128, 128], F32)
        onesr = res_pool.tile([1, 128], F32)

        for k in range(KD):
            nc.gpsimd.dma_start(encTt[:, k, :], encT[k * 128:(k + 1) * 128, :])
            nc.gpsimd.dma_start(decTt[:, k, :], decT[k * 128:(k + 1) * 128, :])
        enc_r = enc.rearrange("(g p) d -> p g d", p=128)
        GJ = JT // 4
        for g in range(4):
            nc.gpsimd.dma_start(encS[:, g * GJ:(g + 1) * GJ, :],
                                enc_r[:, g * GJ:(g + 1) * GJ, :])
        masks.make_identity(nc, identf[:])
        nc.vector.memset(onesr[:], 1.0)

        state = None
        for w in range(WB + 1):
            cur = None
            if w < WB:
                tsl = slice(w * 512, (w + 1) * 512)
                # mm1: s[e-tile, t-chunk], evacuated to SBUF in mm2 dtype;
                # fold-max per e-tile as it lands.
                p = work.tile([128, JT, 512], F32, tag="p", bufs=2)
                fmax = work.tile([128, 512], F32, tag="fmax")
                for j in range(JT):
                    psum_s = ps_s.tile([128, 512], F32, tag="s")
                    for k in range(KD):
                        nc.tensor.matmul(
                            psum_s[:],
                            encTt[:, k, j * 128:(j + 1) * 128],
                            decTt[:, k, tsl],
                            start=(k == 0),
                            stop=(k == KD - 1),
                        )
                    nc.scalar.copy(p[:, j, :], psum_s[:])
                    if j == 0:
                        nc.vector.tensor_copy(fmax[:], psum_s[:])
                    else:
                        nc.vector.tensor_max(fmax[:], fmax[:], psum_s[:])
                # True per-t (row) max: transpose folded-max chunks on the
                # PE, reduce along free (negated), transpose back to a
                # [1,512] row, then DMA-broadcast across partitions.
                nmcol = small.tile([128, 4], F32, tag="nmcol")
                for m in range(4):
                    ptm = ps_m.tile([128, 128], F32, tag="pm")
                    nc.tensor.transpose(ptm[:], fmax[:, m * 128:(m + 1) * 128],
                                        identf[:])
                    nc.vector.reduce_max(out=nmcol[:, m:m + 1], in_=ptm[:],
                                         axis=X, negate=True)
                negm_row = small.tile([1, 512], F32, tag="negmrow")
                for m in range(4):
                    ptr = ps_m.tile([128, 128], F32, tag="pm")
                    nc.tensor.transpose(ptr[:1, :], nmcol[:, m:m + 1], identf[:])
                    nc.scalar.copy(negm_row[:, m * 128:(m + 1) * 128], ptr[:1, :])
                nmb = ps_m.tile([128, 512], F32, tag="nmb")
                nc.tensor.matmul(nmb[:], onesr[:], negm_row[:], start=True, stop=True)
                if dbg is not None:
                    nc.gpsimd.dma_start(dbg[w, 0, :], nmcol[:, 0])
                # subtract row max (add its negation) in fp32, exp into a
                # separate mm2-dtype tile (same pool tag: slots cycle)
                pe_ = work.tile([128, JT, 512], mm2_dt, tag="p", bufs=2)
                for j in range(JT):
                    nc.vector.tensor_add(p[:, j, :], p[:, j, :], nmb[:])
                    nc.scalar.activation(out=pe_[:, j, :], in_=p[:, j, :],
                                         func=Exp, bias=0.0, scale=1.0)
                cur = (pe_, w)
            if state is not None:
                ppv, rzc, wp = state
                for m in range(4):
                    psum_c = ps_c.tile([128, Dd], F32, tag="c")
                    msl = slice(m * 128, (m + 1) * 128)
                    for j in range(JT):
                        nc.tensor.matmul(psum_c[:], ppv[:, j, msl], encS[:, j, :],
                                         start=(j == 0), stop=(j == JT - 1))
                    c = work.tile([128, Dd], F32, tag="c_sb")
                    nc.vector.tensor_scalar_mul(c[:], psum_c[:], rzc[:, m:m + 1])
                    nc.gpsimd.dma_start(out[wp * 512 + m * 128:wp * 512 + (m + 1) * 128, :],
                                        c[:])

            if cur is not None:
                p2, w_ = cur
                # Z path emitted after mm2(w-1): fold-add over exp'd tiles,
                # transpose chunks, free-axis reduce, reciprocal.
                fsum = work.tile([128, 512], F32, tag="fsum")
                for j in range(JT):
                    if j == 0:
                        nc.vector.tensor_copy(fsum[:], p2[:, 0, :])
                    else:
                        nc.vector.tensor_add(fsum[:], fsum[:], p2[:, j, :])
                zc = small.tile([128, 4], F32, tag="zc")
                for m in range(4):
                    pt = ps_m.tile([128, 128], F32, tag="pm")
                    nc.tensor.transpose(pt[:], fsum[:, m * 128:(m + 1) * 128],
                                        identf[:])
                    nc.vector.reduce_sum(out=zc[:, m:m + 1], in_=pt[:], axis=X)
                rzc = small.tile([128, 4], F32, tag="rzc")
                nc.vector.reciprocal(rzc[:], zc[:])
                cur = (p2, rzc, w_)

            state = cur


PLAN = "A"
DEBUG = False


def build(E=S_ENC, T=S_DEC, Dd=D, mm1_dt=MM1_DT, mm2_dt=MM2_DT):
    nc = bass.Bass("TRN2", target_bir_lowering=False, debug=False)
    encT = nc.dram_tensor("encT", [Dd, E], mm1_dt, kind="ExternalInput").ap()
    decT = nc.dram_tensor("decT", [Dd, T], mm1_dt, kind="ExternalInput").ap()
    enc = nc.dram_tensor("enc", [E, Dd], mm2_dt, kind="ExternalInput").ap()
    out = nc.dram_tensor("out", [T, Dd], F32, kind="ExternalOutput").ap()
    dbg = None
    if DEBUG:
        dbg = nc.dram_tensor("dbg", [T // 512, 2, 128], F32, kind="ExternalOutput").ap()
    body = attention_body_b if PLAN == "B" else attention_body
    with tile.TileContext(nc) as tc:
        body(tc, out, encT, decT, enc, E, T, Dd, mm1_dt, mm2_dt, dbg)
    _split_multi_waits(nc)
    return nc


def make_in_maps(enc_output, dec_output):
    enc_output = np.asarray(enc_output, dtype=np.float32)
    dec_output = np.asarray(dec_output, dtype=np.float32)
    in_maps = []
    for b in range(B):
        in_maps.append({
            "encT": np.ascontiguousarray(enc_output[b].T),
            "decT": np.ascontiguousarray(dec_output[b].T),
            "enc": np.ascontiguousarray(enc_output[b]),
        })
    return in_maps


_nc_cache = {}


def _get_nc():
    key = (MM1_DT, MM2_DT)
    if key not in _nc_cache:
        _nc_cache[key] = build()
    return _nc_cache[key]


def kernel(enc_output, dec_output):
    nc = _get_nc()
    in_maps = make_in_maps(enc_output, dec_output)
    last_err = None
    for _attempt in range(3):
        try:
            res = run_bass_kernel_spmd(nc, in_maps, list(range(N_CORES)))
            return np.stack([res.results[b]["out"] for b in range(B)])
        except Exception as e:  # transient device wedge -> retry
            last_err = e
    raise last_err


# revision 33
# speedup vs baseline: 1.2526x; 1.2526x over previous
"""Cross-attention kernel for TRN2 (8 NeuronCores, data-parallel over batch).

Problem (per batch element b):
    s[e,t] = sum_d enc[b,e,d] * dec[b,t,d]
    a      = softmax(s, axis=e)
    out[b,t,d] = sum_e a[e,t] * enc[b,e,d]

Per-core layout (B=8 -> one batch element per core):
  - mm1 computes s in [t_block=128, e] layout: lhsT = decT (d-major), rhs =
    encT (d-major); contraction over d on the PE partition axis.
  - softmax along the free axis: DVE reduce_max(negate) -> ACT exp with
    per-partition bias and accumulated row sum Z.
  - p is transposed 128x128-wise on the PE (identity matmul) so the second
    matmul can contract over e; mm2: lhsT = pT, rhs = enc (natural layout).
  - 1/Z is applied during PSUM evacuation of mm2 (per-partition scalar mul).

Host side transposes enc/dec once (numpy) so the device never transposes
inputs.
"""

import numpy as np

import concourse.bass as bass
import concourse.tile as tile
from concourse import masks, mybir
from concourse.bass_utils import run_bass_kernel_spmd

F32 = mybir.dt.float32
F32R = mybir.dt.float32r

B, S_ENC, S_DEC, D = 8, 2048, 2048, 512
N_CORES = 8

# Matmul input precision knobs (F32 = exact, F32R = ~1e-4, 4x faster rows)
MM1_DT = F32R
MM2_DT = F32R


def _split_multi_waits(nc):
    """This walrus build rejects any instruction with >1 sync wait. Hoist
    surplus waits onto single-wait same-engine NOPs placed just before."""
    for f in nc.m.functions:
        for bb in f.blocks:
            new_list = []
            changed = False
            for inst in bb.instructions:
                si = inst.sync_info
                waits = list(si.on_wait) if si and si.on_wait else []
                if len(waits) > 1:
                    changed = True
                    for w in waits[:-1]:
                        nop = mybir.InstNoOp(
                            name=nc.get_next_instruction_name(),
                            engine=inst.engine,
                            sync_info=mybir.SyncInfo(on_wait=[w], on_update=[]),
                            bass_nofuse=True,
                        )
                        nc.register_instruction(nop, overwrite=True)
                        new_list.append(nop)
                    si.on_wait = waits[-1:]
                new_list.append(inst)
            if changed:
                bb.instructions = new_list


def attention_body(tc, out, encT, decT, enc, E, T, Dd, mm1_dt, mm2_dt, dbg=None):
    nc = tc.nc
    KD = Dd // 128   # d-tiles (contraction of mm1)
    NE = E // 512    # e-chunks of mm1 output (psum bank-sized)
    JT = E // 128    # e-tiles (contraction of mm2 / transposes)
    TB = T // 128    # t row-blocks
    Exp = mybir.ActivationFunctionType.Exp
    X = mybir.AxisListType.X

    with (
        tc.tile_pool(name="resident", bufs=1) as res_pool,
        tc.tile_pool(name="work", bufs=2) as work,
        tc.tile_pool(name="ps_s", bufs=1, space="PSUM") as ps_s,
        tc.tile_pool(name="ps_t", bufs=2, space="PSUM") as ps_t,
        tc.tile_pool(name="ps_c", bufs=2, space="PSUM") as ps_c,
    ):
        encTt = res_pool.tile([128, KD, E], mm1_dt)
        decTt = res_pool.tile([128, KD, T], mm1_dt)
        encS = res_pool.tile([128, JT, Dd], mm2_dt)
        ident = res_pool.tile([128, 128], mm2_dt)

        # Inputs are declared in the matmul dtype directly (f32r binds fp32
        # bits; the PE rounds internally) -> plain DMAs, no rounding casts.
        # Load order matters for the startup ramp: mm1 block 0 needs ALL of
        # encT but only the first t-columns of decT; encS is needed only by
        # mm2 (one softmax later). So: encT, then decT in t-chunks
        # (earliest first), then encS.
        for k in range(KD):
            nc.gpsimd.dma_start(encTt[:, k, :], encT[k * 128:(k + 1) * 128, :])
        TC = T // 4
        for c_ in range(4):
            for k in range(KD):
                nc.gpsimd.dma_start(decTt[:, k, c_ * TC:(c_ + 1) * TC],
                                    decT[k * 128:(k + 1) * 128, c_ * TC:(c_ + 1) * TC])
        enc_r = enc.rearrange("(g p) d -> p g d", p=128)
        GJ = JT // 4
        for g in range(4):
            nc.gpsimd.dma_start(encS[:, g * GJ:(g + 1) * GJ, :],
                                enc_r[:, g * GJ:(g + 1) * GJ, :])
        if mm2_dt == F32:
            masks.make_identity(nc, ident[:])
        else:
            identf = res_pool.tile([128, 128], F32)
            masks.make_identity(nc, identf[:])
            nc.vector.tensor_copy(ident[:], identf[:])

        state = None
        for tb in range(TB + 1):
            cur = None
            if tb < TB:
                # mm1: s[t_block, e] accumulated over d; n-outer so psum bank
                # n is complete after its KD matmuls -> partial max overlaps.
                psum_s = ps_s.tile([128, E], F32, tag="s")
                pmax = work.tile([128, NE], F32, tag="pmax")
                for k in range(KD):
                    for n in range(NE):
                        nc.tensor.matmul(
                            psum_s[:, n * 512:(n + 1) * 512],
                            decTt[:, k, tb * 128:(tb + 1) * 128],
                            encTt[:, k, n * 512:(n + 1) * 512],
                            start=(k == 0),
                            stop=(k == KD - 1),
                        )
                for n in range(NE):
                    nc.vector.reduce_max(out=pmax[:, n:n + 1],
                                         in_=psum_s[:, n * 512:(n + 1) * 512],
                                         axis=X)
                negm = work.tile([128, 1], F32, tag="negm")
                nc.vector.reduce_max(out=negm[:], in_=pmax[:], axis=X, negate=True)
                p = work.tile([128, E], mm2_dt, tag="p")
                zp = work.tile([128, NE], F32, tag="zp")
                for n in range(NE):
                    nc.scalar.activation(out=p[:, n * 512:(n + 1) * 512],
                                         in_=psum_s[:, n * 512:(n + 1) * 512],
                                         func=Exp, bias=negm[:], scale=1.0,
                                         accum_out=zp[:, n:n + 1])
                z = work.tile([128, 1], F32, tag="z")
                nc.vector.reduce_sum(out=z[:], in_=zp[:], axis=X)
                rz = work.tile([128, 1], F32, tag="rz")
                nc.vector.reciprocal(rz[:], z[:])
                cur = (p, rz, tb)

            if state is not None:
                pp, rz, tbp = state
                pT = work.tile([128, JT, 128], mm2_dt, tag="pT")
                for j in range(JT):
                    pst = ps_t.tile([128, 128], mm2_dt, tag="pt")
                    nc.tensor.transpose(pst[:], pp[:, j * 128:(j + 1) * 128], ident[:])
                    if j % 2 == 0:
                        nc.scalar.copy(pT[:, j, :], pst[:])
                    else:
                        nc.vector.tensor_copy(pT[:, j, :], pst[:])
                psum_c = ps_c.tile([128, Dd], F32, tag="c")
                for j in range(JT):
                    nc.tensor.matmul(psum_c[:], pT[:, j, :], encS[:, j, :],
                                     start=(j == 0), stop=(j == JT - 1))
                c = work.tile([128, Dd], F32, tag="c_sb")
                nc.vector.tensor_scalar_mul(c[:], psum_c[:], rz[:])
                nc.gpsimd.dma_start(out[tbp * 128:(tbp + 1) * 128, :], c[:])

            state = cur


def attention_body_b(tc, out, encT, decT, enc, E, T, Dd, mm1_dt, mm2_dt, dbg=None):
    """Plan B (unused alternative): scores in [e,t] layout, no probability
    transposes. True per-row max via PE transposes of the folded max + a
    rank-1 ones matmul broadcast; Z via fold-add + PE transpose + reduce.
    Measured slower than Plan A on this HW (227.5us vs 224.8us): every
    matmul changes weights and the folds load the DVE heavily.
    """
    nc = tc.nc
    KD = Dd // 128   # d-tiles (contraction of mm1)
    JT = E // 128    # e-tiles
    WB = T // 512    # t column-blocks
    Exp = mybir.ActivationFunctionType.Exp
    X = mybir.AxisListType.X

    with (
        tc.tile_pool(name="resident", bufs=1) as res_pool,
        tc.tile_pool(name="work", bufs=2) as work,
        tc.tile_pool(name="small", bufs=2) as small,
        tc.tile_pool(name="ps_s", bufs=4, space="PSUM") as ps_s,
        tc.tile_pool(name="ps_m", bufs=1, space="PSUM") as ps_m,
        tc.tile_pool(name="ps_c", bufs=2, space="PSUM") as ps_c,
    ):
        encTt = res_pool.tile([128, KD, E], mm1_dt)
        decTt = res_pool.tile([128, KD, T], mm1_dt)
        encS = res_pool.tile([128, JT, Dd], mm2_dt)
        identf = res_pool.tile([128, 128], F32)
        onesr = res_pool.tile([1, 128], F32)

        for k in range(KD):
            nc.gpsimd.dma_start(encTt[:, k, :], encT[k * 128:(k + 1) * 128, :])
            nc.gpsimd.dma_start(decTt[:, k, :], decT[k * 128:(k + 1) * 128, :])
        enc_r = enc.rearrange("(g p) d -> p g d", p=128)
        GJ = JT // 4
        for g in range(4):
            nc.gpsimd.dma_start(encS[:, g * GJ:(g + 1) * GJ, :],
                                enc_r[:, g * GJ:(g + 1) * GJ, :])
        masks.make_identity(nc, identf[:])
        nc.vector.memset(onesr[:], 1.0)

        state = None
        for w in range(WB + 1):
            cur = None
            if w < WB:
                tsl = slice(w * 512, (w + 1) * 512)
                # mm1: s[e-tile, t-chunk], evacuated to SBUF in mm2 dtype;
                # fold-max per e-tile as it lands.
                p = work.tile([128, JT, 512], F32, tag="p", bufs=2)
                fmax = work.tile([128, 512], F32, tag="fmax")
                for j in range(JT):
                    psum_s = ps_s.tile([128, 512], F32, tag="s")
                    for k in range(KD):
                        nc.tensor.matmul(
                            psum_s[:],
                            encTt[:, k, j * 128:(j + 1) * 128],
                            decTt[:, k, tsl],
                            start=(k == 0),
                            stop=(k == KD - 1),
                        )
                    nc.scalar.copy(p[:, j, :], psum_s[:])
                    if j == 0:
                        nc.vector.tensor_copy(fmax[:], psum_s[:])
                    else:
                        nc.vector.tensor_max(fmax[:], fmax[:], psum_s[:])
                # True per-t (row) max: transpose folded-max chunks on the
                # PE, reduce along free (negated), transpose back to a
                # [1,512] row, then DMA-broadcast across partitions.
                nmcol = small.tile([128, 4], F32, tag="nmcol")
                for m in range(4):
                    ptm = ps_m.tile([128, 128], F32, tag="pm")
                    nc.tensor.transpose(ptm[:], fmax[:, m * 128:(m + 1) * 128],
                                        identf[:])
                    nc.vector.reduce_max(out=nmcol[:, m:m + 1], in_=ptm[:],
                                         axis=X, negate=True)
                negm_row = small.tile([1, 512], F32, tag="negmrow")
                for m in range(4):
                    ptr = ps_m.tile([128, 128], F32, tag="pm")
                    nc.tensor.transpose(ptr[:1, :], nmcol[:, m:m + 1], identf[:])
                    nc.scalar.copy(negm_row[:, m * 128:(m + 1) * 128], ptr[:1, :])
                nmb = ps_m.tile([128, 512], F32, tag="nmb")
                nc.tensor.matmul(nmb[:], onesr[:], negm_row[:], start=True, stop=True)
                if dbg is not None:
                    nc.gpsimd.dma_start(dbg[w, 0, :], nmcol[:, 0])
                # subtract row max (add its negation) in fp32, exp into a
                # separate mm2-dtype tile (same pool tag: slots cycle)
                pe_ = work.tile([128, JT, 512], mm2_dt, tag="p", bufs=2)
                for j in range(JT):
                    nc.vector.tensor_add(p[:, j, :], p[:, j, :], nmb[:])
                    nc.scalar.activation(out=pe_[:, j, :], in_=p[:, j, :],
                                         func=Exp, bias=0.0, scale=1.0)
                cur = (pe_, w)
            if state is not None:
                ppv, rzc, wp = state
                for m in range(4):
                    psum_c = ps_c.tile([128, Dd], F32, tag="c")
                    msl = slice(m * 128, (m + 1) * 128)
                    for j in range(JT):
                        nc.tensor.matmul(psum_c[:], ppv[:, j, msl], encS[:, j, :],
                                         start=(j == 0), stop=(j == JT - 1))
                    c = work.tile([128, Dd], F32, tag="c_sb")
                    nc.vector.tensor_scalar_mul(c[:], psum_c[:], rzc[:, m:m + 1])
                    nc.gpsimd.dma_start(out[wp * 512 + m * 128:wp * 512 + (m + 1) * 128, :],
                                        c[:])

            if cur is not None:
                p2, w_ = cur
                # Z path emitted after mm2(w-1): fold-add over exp'd tiles,
                # transpose chunks, free-axis reduce, reciprocal.
                fsum = work.tile([128, 512], F32, tag="fsum")
                for j in range(JT):
                    if j == 0:
                        nc.vector.tensor_copy(fsum[:], p2[:, 0, :])
                    else:
                        nc.vector.tensor_add(fsum[:], fsum[:], p2[:, j, :])
                zc = small.tile([128, 4], F32, tag="zc")
                for m in range(4):
                    pt = ps_m.tile([128, 128], F32, tag="pm")
                    nc.tensor.transpose(pt[:], fsum[:, m * 128:(m + 1) * 128],
                                        identf[:])
                    nc.vector.reduce_sum(out=zc[:, m:m + 1], in_=pt[:], axis=X)
                rzc = small.tile([128, 4], F32, tag="rzc")
                nc.vector.reciprocal(rzc[:], zc[:])
                cur = (p2, rzc, w_)

            state = cur


PLAN = "A"
DEBUG = False


def build(E=S_ENC, T=S_DEC, Dd=D, mm1_dt=MM1_DT, mm2_dt=MM2_DT):
    nc = bass.Bass("TRN2", target_bir_lowering=False, debug=False)
    encT = nc.dram_tensor("encT", [Dd, E], mm1_dt, kind="ExternalInput").ap()
    decT = nc.dram_tensor("decT", [Dd, T], mm1_dt, kind="ExternalInput").ap()
    enc = nc.dram_tensor("enc", [E, Dd], mm2_dt, kind="ExternalInput").ap()
    out = nc.dram_tensor("out", [T, Dd], F32, kind="ExternalOutput").ap()
    dbg = None
    if DEBUG:
        dbg = nc.dram_tensor("dbg", [T // 512, 2, 128], F32, kind="ExternalOutput").ap()
    body = attention_body_b if PLAN == "B" else attention_body
    with tile.TileContext(nc) as tc:
        body(tc, out, encT, decT, enc, E, T, Dd, mm1_dt, mm2_dt, dbg)
    _split_multi_waits(nc)
    return nc


def make_in_maps(enc_output, dec_output):
    enc_output = np.asarray(enc_output, dtype=np.float32)
    dec_output = np.asarray(dec_output, dtype=np.float32)
    in_maps = []
    for b in range(B):
        in_maps.append({
            "encT": np.ascontiguousarray(enc_output[b].T),
            "decT": np.ascontiguousarray(dec_output[b].T),
            "enc": np.ascontiguousarray(enc_output[b]),
        })
    return in_maps


_nc_cache = {}


def _get_nc():
    key = (MM1_DT, MM2_DT)
    if key not in _nc_cache:
        _nc_cache[key] = build()
    return _nc_cache[key]


def kernel(enc_output, dec_output):
    nc = _get_nc()
    in_maps = make_in_maps(enc_output, dec_output)
    last_err = None
    for _attempt in range(3):
        try:
            res = run_bass_kernel_spmd(nc, in_maps, list(range(N_CORES)))
            return np.stack([res.results[b]["out"] for b in range(B)])
        except Exception as e:  # transient device wedge -> retry
            last_err = e
    raise last_err


# revision 36
# speedup vs baseline: 1.2565x; 1.0031x over previous
"""Cross-attention kernel for TRN2 (8 NeuronCores, data-parallel over batch).

Problem (per batch element b):
    s[e,t] = sum_d enc[b,e,d] * dec[b,t,d]
    a      = softmax(s, axis=e)
    out[b,t,d] = sum_e a[e,t] * enc[b,e,d]

Per-core layout (B=8 -> one batch element per core):
  - mm1 computes s in [t_block=128, e] layout: lhsT = decT (d-major), rhs =
    encT (d-major); contraction over d on the PE partition axis.
  - softmax along the free axis: DVE reduce_max(negate) -> ACT exp with
    per-partition bias and accumulated row sum Z.
  - p is transposed 128x128-wise on the PE (identity matmul) so the second
    matmul can contract over e; mm2: lhsT = pT, rhs = enc (natural layout).
  - 1/Z is applied during PSUM evacuation of mm2 (per-partition scalar mul).

Host side transposes enc/dec once (numpy) so the device never transposes
inputs.
"""

import numpy as np

import concourse.bass as bass
import concourse.tile as tile
from concourse import masks, mybir
from concourse.bass_utils import run_bass_kernel_spmd

F32 = mybir.dt.float32
F32R = mybir.dt.float32r


def _fast_drain_and_barrier(self, tick_clock, wait_clock):
    # Tile tail without the second all-engine barrier: NEFF completion
    # already waits for every engine queue to drain, and the gpsimd sem/dma
    # clears are ordered within the gpsimd queue, so re-execution still sees
    # cleared semaphores. Saves a few us of fixed tail per execution.
    from concourse.vector_clock import ScopedClock
    nc = self.nc
    drain_inst = nc.sync.drain()
    wait_clock.add_sem_waits(drain_inst.ins,
                             ScopedClock({None: tick_clock.global_clock}))
    nc.all_engine_barrier()
    popped = nc._tile_sem_poison_stack.pop()
    assert popped is self._sem_poison
    nc.clear_and_free_semaphores(list(self.sems.allocated().values()))


tile.TileContext._drain_and_barrier = _fast_drain_and_barrier

B, S_ENC, S_DEC, D = 8, 2048, 2048, 512
N_CORES = 8

# Matmul input precision knobs (F32 = exact, F32R = ~1e-4, 4x faster rows)
MM1_DT = F32R
MM2_DT = F32R


def _split_multi_waits(nc):
    """This walrus build rejects any instruction with >1 sync wait. Hoist
    surplus waits onto single-wait same-engine NOPs placed just before."""
    for f in nc.m.functions:
        for bb in f.blocks:
            new_list = []
            changed = False
            for inst in bb.instructions:
                si = inst.sync_info
                waits = list(si.on_wait) if si and si.on_wait else []
                if len(waits) > 1:
                    changed = True
                    for w in waits[:-1]:
                        nop = mybir.InstNoOp(
                            name=nc.get_next_instruction_name(),
                            engine=inst.engine,
                            sync_info=mybir.SyncInfo(on_wait=[w], on_update=[]),
                            bass_nofuse=True,
                        )
                        nc.register_instruction(nop, overwrite=True)
                        new_list.append(nop)
                    si.on_wait = waits[-1:]
                new_list.append(inst)
            if changed:
                bb.instructions = new_list


def attention_body(tc, out, encT, decT, enc, E, T, Dd, mm1_dt, mm2_dt, dbg=None):
    nc = tc.nc
    KD = Dd // 128   # d-tiles (contraction of mm1)
    NE = E // 512    # e-chunks of mm1 output (psum bank-sized)
    JT = E // 128    # e-tiles (contraction of mm2 / transposes)
    TB = T // 128    # t row-blocks
    Exp = mybir.ActivationFunctionType.Exp
    X = mybir.AxisListType.X

    with (
        tc.tile_pool(name="resident", bufs=1) as res_pool,
        tc.tile_pool(name="work", bufs=2) as work,
        tc.tile_pool(name="ps_s", bufs=1, space="PSUM") as ps_s,
        tc.tile_pool(name="ps_t", bufs=2, space="PSUM") as ps_t,
        tc.tile_pool(name="ps_c", bufs=2, space="PSUM") as ps_c,
    ):
        encTt = res_pool.tile([128, KD, E], mm1_dt)
        decTt = res_pool.tile([128, KD, T], mm1_dt)
        encS = res_pool.tile([128, JT, Dd], mm2_dt)
        ident = res_pool.tile([128, 128], mm2_dt)

        # Inputs are declared in the matmul dtype directly (f32r binds fp32
        # bits; the PE rounds internally) -> plain DMAs, no rounding casts.
        # Load order matters for the startup ramp: mm1 block 0 needs ALL of
        # encT but only the first t-columns of decT; encS is needed only by
        # mm2 (one softmax later). So: encT, then decT in t-chunks
        # (earliest first), then encS.
        for k in range(KD):
            nc.gpsimd.dma_start(encTt[:, k, :], encT[k * 128:(k + 1) * 128, :])
        TC = T // 4
        for c_ in range(4):
            for k in range(KD):
                nc.gpsimd.dma_start(decTt[:, k, c_ * TC:(c_ + 1) * TC],
                                    decT[k * 128:(k + 1) * 128, c_ * TC:(c_ + 1) * TC])
        enc_r = enc.rearrange("(g p) d -> p g d", p=128)
        GJ = JT // 4
        for g in range(4):
            nc.gpsimd.dma_start(encS[:, g * GJ:(g + 1) * GJ, :],
                                enc_r[:, g * GJ:(g + 1) * GJ, :])
        if mm2_dt == F32:
            masks.make_identity(nc, ident[:])
        else:
            identf = res_pool.tile([128, 128], F32)
            masks.make_identity(nc, identf[:])
            nc.vector.tensor_copy(ident[:], identf[:])

        state = None
        for tb in range(TB + 1):
            cur = None
            if tb < TB:
                # mm1: s[t_block, e] accumulated over d; n-outer so psum bank
                # n is complete after its KD matmuls -> partial max overlaps.
                psum_s = ps_s.tile([128, E], F32, tag="s")
                pmax = work.tile([128, NE], F32, tag="pmax")
                for k in range(KD):
                    for n in range(NE):
                        nc.tensor.matmul(
                            psum_s[:, n * 512:(n + 1) * 512],
                            decTt[:, k, tb * 128:(tb + 1) * 128],
                            encTt[:, k, n * 512:(n + 1) * 512],
                            start=(k == 0),
                            stop=(k == KD - 1),
                        )
                for n in range(NE):
                    nc.vector.reduce_max(out=pmax[:, n:n + 1],
                                         in_=psum_s[:, n * 512:(n + 1) * 512],
                                         axis=X)
                negm = work.tile([128, 1], F32, tag="negm")
                nc.vector.reduce_max(out=negm[:], in_=pmax[:], axis=X, negate=True)
                p = work.tile([128, E], mm2_dt, tag="p")
                zp = work.tile([128, NE], F32, tag="zp")
                for n in range(NE):
                    nc.scalar.activation(out=p[:, n * 512:(n + 1) * 512],
                                         in_=psum_s[:, n * 512:(n + 1) * 512],
                                         func=Exp, bias=negm[:], scale=1.0,
                                         accum_out=zp[:, n:n + 1])
                z = work.tile([128, 1], F32, tag="z")
                nc.vector.reduce_sum(out=z[:], in_=zp[:], axis=X)
                rz = work.tile([128, 1], F32, tag="rz")
                nc.vector.reciprocal(rz[:], z[:])
                cur = (p, rz, tb)

            if state is not None:
                pp, rz, tbp = state
                pT = work.tile([128, JT, 128], mm2_dt, tag="pT")
                for j in range(JT):
                    pst = ps_t.tile([128, 128], mm2_dt, tag="pt")
                    nc.tensor.transpose(pst[:], pp[:, j * 128:(j + 1) * 128], ident[:])
                    if j % 2 == 0:
                        nc.scalar.copy(pT[:, j, :], pst[:])
                    else:
                        nc.vector.tensor_copy(pT[:, j, :], pst[:])
                psum_c = ps_c.tile([128, Dd], F32, tag="c")
                for j in range(JT):
                    nc.tensor.matmul(psum_c[:], pT[:, j, :], encS[:, j, :],
                                     start=(j == 0), stop=(j == JT - 1))
                c = work.tile([128, Dd], F32, tag="c_sb")
                nc.vector.tensor_scalar_mul(c[:], psum_c[:], rz[:])
                nc.gpsimd.dma_start(out[tbp * 128:(tbp + 1) * 128, :], c[:])

            state = cur


def attention_body_b(tc, out, encT, decT, enc, E, T, Dd, mm1_dt, mm2_dt, dbg=None):
    """Plan B (unused alternative): scores in [e,t] layout, no probability
    transposes. True per-row max via PE transposes of the folded max + a
    rank-1 ones matmul broadcast; Z via fold-add + PE transpose + reduce.
    Measured slower than Plan A on this HW (227.5us vs 224.8us): every
    matmul changes weights and the folds load the DVE heavily.
    """
    nc = tc.nc
    KD = Dd // 128   # d-tiles (contraction of mm1)
    JT = E // 128    # e-tiles
    WB = T // 512    # t column-blocks
    Exp = mybir.ActivationFunctionType.Exp
    X = mybir.AxisListType.X

    with (
        tc.tile_pool(name="resident", bufs=1) as res_pool,
        tc.tile_pool(name="work", bufs=2) as work,
        tc.tile_pool(name="small", bufs=2) as small,
        tc.tile_pool(name="ps_s", bufs=4, space="PSUM") as ps_s,
        tc.tile_pool(name="ps_m", bufs=1, space="PSUM") as ps_m,
        tc.tile_pool(name="ps_c", bufs=2, space="PSUM") as ps_c,
    ):
        encTt = res_pool.tile([128, KD, E], mm1_dt)
        decTt = res_pool.tile([128, KD, T], mm1_dt)
        encS = res_pool.tile([128, JT, Dd], mm2_dt)
        identf = res_pool.tile([128, 128], F32)
        onesr = res_pool.tile([1, 128], F32)

        for k in range(KD):
            nc.gpsimd.dma_start(encTt[:, k, :], encT[k * 128:(k + 1) * 128, :])
            nc.gpsimd.dma_start(decTt[:, k, :], decT[k * 128:(k + 1) * 128, :])
        enc_r = enc.rearrange("(g p) d -> p g d", p=128)
        GJ = JT // 4
        for g in range(4):
            nc.gpsimd.dma_start(encS[:, g * GJ:(g + 1) * GJ, :],
                                enc_r[:, g * GJ:(g + 1) * GJ, :])
        masks.make_identity(nc, identf[:])
        nc.vector.memset(onesr[:], 1.0)

        state = None
        for w in range(WB + 1):
            cur = None
            if w < WB:
                tsl = slice(w * 512, (w + 1) * 512)
                # mm1: s[e-tile, t-chunk], evacuated to SBUF in mm2 dtype;
                # fold-max per e-tile as it lands.
                p = work.tile([128, JT, 512], F32, tag="p", bufs=2)
                fmax = work.tile([128, 512], F32, tag="fmax")
                for j in range(JT):
                    psum_s = ps_s.tile([128, 512], F32, tag="s")
                    for k in range(KD):
                        nc.tensor.matmul(
                            psum_s[:],
                            encTt[:, k, j * 128:(j + 1) * 128],
                            decTt[:, k, tsl],
                            start=(k == 0),
                            stop=(k == KD - 1),
                        )
                    nc.scalar.copy(p[:, j, :], psum_s[:])
                    if j == 0:
                        nc.vector.tensor_copy(fmax[:], psum_s[:])
                    else:
                        nc.vector.tensor_max(fmax[:], fmax[:], psum_s[:])
                # True per-t (row) max: transpose folded-max chunks on the
                # PE, reduce along free (negated), transpose back to a
                # [1,512] row, then DMA-broadcast across partitions.
                nmcol = small.tile([128, 4], F32, tag="nmcol")
                for m in range(4):
                    ptm = ps_m.tile([128, 128], F32, tag="pm")
                    nc.tensor.transpose(ptm[:], fmax[:, m * 128:(m + 1) * 128],
                                        identf[:])
                    nc.vector.reduce_max(out=nmcol[:, m:m + 1], in_=ptm[:],
                                         axis=X, negate=True)
                negm_row = small.tile([1, 512], F32, tag="negmrow")
                for m in range(4):
                    ptr = ps_m.tile([128, 128], F32, tag="pm")
                    nc.tensor.transpose(ptr[:1, :], nmcol[:, m:m + 1], identf[:])
                    nc.scalar.copy(negm_row[:, m * 128:(m + 1) * 128], ptr[:1, :])
                nmb = ps_m.tile([128, 512], F32, tag="nmb")
                nc.tensor.matmul(nmb[:], onesr[:], negm_row[:], start=True, stop=True)
                if dbg is not None:
                    nc.gpsimd.dma_start(dbg[w, 0, :], nmcol[:, 0])
                # subtract row max (add its negation) in fp32, exp into a
                # separate mm2-dtype tile (same pool tag: slots cycle)
                pe_ = work.tile([128, JT, 512], mm2_dt, tag="p", bufs=2)
                for j in range(JT):
                    nc.vector.tensor_add(p[:, j, :], p[:, j, :], nmb[:])
                    nc.scalar.activation(out=pe_[:, j, :], in_=p[:, j, :],
                                         func=Exp, bias=0.0, scale=1.0)
                cur = (pe_, w)
            if state is not None:
                ppv, rzc, wp = state
                for m in range(4):
                    psum_c = ps_c.tile([128, Dd], F32, tag="c")
                    msl = slice(m * 128, (m + 1) * 128)
                    for j in range(JT):
                        nc.tensor.matmul(psum_c[:], ppv[:, j, msl], encS[:, j, :],
                                         start=(j == 0), stop=(j == JT - 1))
                    c = work.tile([128, Dd], F32, tag="c_sb")
                    nc.vector.tensor_scalar_mul(c[:], psum_c[:], rzc[:, m:m + 1])
                    nc.gpsimd.dma_start(out[wp * 512 + m * 128:wp * 512 + (m + 1) * 128, :],
                                        c[:])

            if cur is not None:
                p2, w_ = cur
                # Z path emitted after mm2(w-1): fold-add over exp'd tiles,
                # transpose chunks, free-axis reduce, reciprocal.
                fsum = work.tile([128, 512], F32, tag="fsum")
                for j in range(JT):
                    if j == 0:
                        nc.vector.tensor_copy(fsum[:], p2[:, 0, :])
                    else:
                        nc.vector.tensor_add(fsum[:], fsum[:], p2[:, j, :])
                zc = small.tile([128, 4], F32, tag="zc")
                for m in range(4):
                    pt = ps_m.tile([128, 128], F32, tag="pm")
                    nc.tensor.transpose(pt[:], fsum[:, m * 128:(m + 1) * 128],
                                        identf[:])
                    nc.vector.reduce_sum(out=zc[:, m:m + 1], in_=pt[:], axis=X)
                rzc = small.tile([128, 4], F32, tag="rzc")
                nc.vector.reciprocal(rzc[:], zc[:])
                cur = (p2, rzc, w_)

            state = cur


PLAN = "A"
DEBUG = False


def build(E=S_ENC, T=S_DEC, Dd=D, mm1_dt=MM1_DT, mm2_dt=MM2_DT):
    nc = bass.Bass("TRN2", target_bir_lowering=False, debug=False)
    encT = nc.dram_tensor("encT", [Dd, E], mm1_dt, kind="ExternalInput").ap()
    decT = nc.dram_tensor("decT", [Dd, T], mm1_dt, kind="ExternalInput").ap()
    enc = nc.dram_tensor("enc", [E, Dd], mm2_dt, kind="ExternalInput").ap()
    out = nc.dram_tensor("out", [T, Dd], F32, kind="ExternalOutput").ap()
    dbg = None
    if DEBUG:
        dbg = nc.dram_tensor("dbg", [T // 512, 2, 128], F32, kind="ExternalOutput").ap()
    body = attention_body_b if PLAN == "B" else attention_body
    with tile.TileContext(nc) as tc:
        body(tc, out, encT, decT, enc, E, T, Dd, mm1_dt, mm2_dt, dbg)
    _split_multi_waits(nc)
    return nc


def make_in_maps(enc_output, dec_output):
    enc_output = np.asarray(enc_output, dtype=np.float32)
    dec_output = np.asarray(dec_output, dtype=np.float32)
    in_maps = []
    for b in range(B):
        in_maps.append({
            "encT": np.ascontiguousarray(enc_output[b].T),
            "decT": np.ascontiguousarray(dec_output[b].T),
            "enc": np.ascontiguousarray(enc_output[b]),
        })
    return in_maps


_nc_cache = {}


def _get_nc():
    key = (MM1_DT, MM2_DT)
    if key not in _nc_cache:
        _nc_cache[key] = build()
    return _nc_cache[key]


def kernel(enc_output, dec_output):
    nc = _get_nc()
    in_maps = make_in_maps(enc_output, dec_output)
    last_err = None
    for _attempt in range(3):
        try:
            res = run_bass_kernel_spmd(nc, in_maps, list(range(N_CORES)))
            return np.stack([res.results[b]["out"] for b in range(B)])
        except Exception as e:  # transient device wedge -> retry
            last_err = e
    raise last_err
